# revision 1
# baseline (speedup 1.0000x reference)
"""Additive (Bahdanau) attention kernel for Trainium2, data-parallel over batch.

Problem shapes (hardcoded per contract): S=128, B=16, T=64, H=256.
  outputs: (S, B, 2H) f32   encoder states
  src_len: (B,)       i64   valid source lengths
  ss:      (T, B, H)  f32   decoder states
  W1 (2H,H), b1 (H), W2 (H,H), b2 (H), v_w (H), v_b (1)

reference:
  wh = outputs @ W1 + b1          -> (B,S,H)
  ws = ss @ W2 + b2               -> (B,T,H)
  scores = tanh(wh[:,None]+ws[:,:,None]) . v_w (+v_b)   -> (B,T,S)
  masked softmax over S, then attn @ outputs -> (B,T,2H)

Sharding: batch across 8 cores (2 batches per core), weights replicated.

Device pipeline per batch (s-outer layout):
  fc1/fc2 on PE (fp16 operands, f32 accum); wh evicted with bias as
  duplicated fp16 pairs (whdup) for the DVE 2x outer-add trick.
  Per s-group (48/48/32 source rows):
    DVE outer-add  big[h, (s, t)] = wh[h,s] + ws[h,t]   (fp16, 2x mode)
    ACT tanh (fp16)
    PE h-reduction: stationary = v_w replicated to 32 cols -> each 8-s chunk
      lands as 32 identical psum rows at col-groups {0,32,64}; contiguous
      [96,1024] psum block evicted with one DVE copy, then DMA-relayout
      into scoresT [s, t].
  exp with per-partition mask bias (-1e30 for s >= src_len) -> e [s, t]
  final: out[t,:512|den] = e.T @ [outputs | ones] on PE; normalize by 1/den.
v_b is omitted (softmax shift-invariant); no max-subtraction (|score| <~ 13).
"""

import os
import numpy as np
from contextlib import ExitStack

import concourse.bass as bass
import concourse.bacc as bacc
import concourse.tile as tile
from concourse import mybir
from concourse.bass_utils import run_bass_kernel_spmd


S, B, T, H = 128, 16, 64, 256
E = 2 * H
NCORES = 8
BPC = B // NCORES          # batches per core
SGROUPS = [48, 48, 32]     # s rows per group (chunks of 8 s x 64 t = 512 cols)
F32, F16 = mybir.dt.float32, mybir.dt.float16

_prog_cache = {}


def build_program():
    nc = bacc.Bacc("TRN2", target_bir_lowering=False, debug=False,
                   num_devices=NCORES)

    outs_d = nc.declare_dram_parameter("outs", [BPC, S, E], F32, isOutput=False)
    outsT_d = nc.declare_dram_parameter("outsT16", [BPC, E, S], F16, isOutput=False)
    ssT_d = nc.declare_dram_parameter("ssT16", [BPC, H, T], F16, isOutput=False)
    w1_d = nc.declare_dram_parameter("w1", [E, H], F16, isOutput=False)
    w2_d = nc.declare_dram_parameter("w2", [H, H], F16, isOutput=False)
    b1_d = nc.declare_dram_parameter("b1", [H], F32, isOutput=False)
    b2_d = nc.declare_dram_parameter("b2", [H], F32, isOutput=False)
    vrep_d = nc.declare_dram_parameter("vrep", [128, 2, 32], F16, isOutput=False)
    maskb_d = nc.declare_dram_parameter("maskb", [BPC, S], F32, isOutput=False)
    slen_d = nc.declare_dram_parameter("slen", [BPC], mybir.dt.int32, isOutput=False)
    out_d = nc.declare_dram_parameter("out", [BPC, T, E], F32, isOutput=True)

    with ExitStack() as ctx:
        tc = ctx.enter_context(tile.TileContext(nc))
        consts = ctx.enter_context(tc.tile_pool(name="consts", bufs=1))
        work = ctx.enter_context(tc.tile_pool(name="work", bufs=2))
        bigp = ctx.enter_context(tc.tile_pool(name="bigp", bufs=6))
        smallp = ctx.enter_context(tc.tile_pool(name="smallp", bufs=2))
        fc_ps = ctx.enter_context(tc.tile_pool(name="fc_ps", bufs=2, space="PSUM"))
        scq_ps = ctx.enter_context(tc.tile_pool(name="scq_ps", bufs=2, space="PSUM"))
        out_ps = ctx.enter_context(tc.tile_pool(name="out_ps", bufs=1, space="PSUM"))

        # ---- constants ----
        ones_sb = consts.tile([128, 1], F32)
        nc.vector.memset(ones_sb, 1.0)
        slen_sb = consts.tile([1, BPC], mybir.dt.int32)
        nc.sync.dma_start(slen_sb, slen_d[:])
        # weights + small constants go on the gpsimd SWDGE queue so they don't
        # serialize behind the activation loads on the sync HWDGE queue
        w1_sb = consts.tile([128, 4, H], F16)  # [p, kchunk, m] = W1[kc*128+p, m]
        nc.gpsimd.dma_start(w1_sb, w1_d.rearrange("(c p) m -> p c m", p=128))
        w2_sb = consts.tile([128, 2, H], F16)
        nc.gpsimd.dma_start(w2_sb, w2_d.rearrange("(c p) m -> p c m", p=128))
        b1_sb = consts.tile([128, 2], F32)  # [p, mhalf] = b1[m*128+p]
        nc.gpsimd.dma_start(b1_sb, b1_d.rearrange("(m p) -> p m", p=128))
        b2_sb = consts.tile([128, 2], F32)
        nc.gpsimd.dma_start(b2_sb, b2_d.rearrange("(m p) -> p m", p=128))
        vrep_sb = consts.tile([128, 2, 32], F16)
        nc.gpsimd.dma_start(vrep_sb, vrep_d[:])
        maskb_sb = consts.tile([128, BPC], F32)  # [s, b] 0 valid / -1e30 invalid
        nc.gpsimd.dma_start(maskb_sb, maskb_d.rearrange("b p -> p b"))

        # ---- phase 1: per-batch prologue (loads, transposes, fc1/fc2) ----
        outs_nat = [None] * BPC
        whdup = [None] * BPC
        wsT = [None] * BPC
        scoresT = [None] * BPC
        rv = [None] * BPC
        for b in range(BPC):
            outs_nat[b] = work.tile([128, E], F32, tag="outs_nat", name=f"outs_nat{b}")  # [s, e]
            nc.sync.dma_start(outs_nat[b], outs_d[b])
            if b == 0 and not os.environ.get("KERNEL_NO_WARM"):
                # PE warmup tied to first data: ~3.4us of f32 matmuls right
                # when PE work begins, so HAM reaches K=8/8 and stays there.
                warm_ps = fc_ps.tile([128, 512], F32, tag="fc")
                for _ in range(2):
                    nc.tensor.matmul(warm_ps, outs_nat[b][:, 0:128],
                                     outs_nat[b], start=True,
                                     stop=True, skip_group_check=True)
            outsT = work.tile([128, 4, 128], F16, tag="outsT")  # [e%128, ec, s]
            nc.sync.dma_start(outsT, outsT_d[b].rearrange("(c p) s -> p c s",
                                                          p=128))

            # fc1 -> whdup[h, 2s+j] (duplicated pairs, +b1, fp16)
            whdup[b] = work.tile([128, 2, 2 * S], F16, tag="whdup", name=f"whdup{b}")
            for m in range(2):
                ps = fc_ps.tile([128, 128], F32, tag="fc")
                for c in range(4):
                    nc.tensor.matmul(ps, w1_sb[:, c, m * 128:(m + 1) * 128],
                                     outsT[:, c, :], start=(c == 0), stop=(c == 3))
                nc.vector.tensor_scalar_add(whdup[b][:, m, 0:2 * S:2], ps,
                                            b1_sb[:, m:m + 1])
                nc.vector.tensor_scalar_add(whdup[b][:, m, 1:2 * S:2], ps,
                                            b1_sb[:, m:m + 1])

            # load pre-transposed ss, fc2 -> wsT[h, t] (+b2, fp16)
            ssT = work.tile([128, 2, T], F16, tag="ssT")    # [h%128, hchunk, t]
            nc.sync.dma_start(ssT, ssT_d[b].rearrange("(c p) t -> p c t", p=128))
            wsT[b] = work.tile([128, 2, T], F16, tag="wsT", name=f"wsT{b}")
            for m in range(2):
                ps = fc_ps.tile([128, T], F32, tag="fc")
                for c in range(2):
                    nc.tensor.matmul(ps, w2_sb[:, c, m * 128:(m + 1) * 128],
                                     ssT[:, c, :], start=(c == 0), stop=(c == 1))
                nc.vector.tensor_scalar_add(wsT[b][:, m, :], ps, b2_sb[:, m:m + 1])

            scoresT[b] = smallp.tile([S, T], F32, tag="scoresT", name=f"scoresT{b}")
            nc.gpsimd.memset(scoresT[b], 0.0)  # guard first-use NaN
            rv[b] = nc.values_load(
                slen_sb[0:1, b:b + 1], min_val=1, max_val=S,
                skip_runtime_bounds_check=True,
                engines=(mybir.EngineType.PE, mybir.EngineType.DVE,
                         mybir.EngineType.Activation))

        # ---- phase 2: big pipeline per (s-group, batch), evict delayed by
        #      one slab so DVE/ACT never block the next slab's add/tanh on
        #      the previous slab's PSUM eviction ----
        def slab_compute(b, sbase, scount, scq):
            """Engine-only work (DVE/ACT/PE) — safe inside tc.If."""
            nch = scount // 8
            big = bigp.tile([128, 2, 64 * SGROUPS[0]], F16, tag="big")
            for m in range(2):
                in0 = (whdup[b][:, m, 2 * sbase:2 * (sbase + scount)]
                       .rearrange("p (s two) -> p s two", two=2)
                       [:, :, None, :].broadcast_to([128, scount, 32, 2]))
                in1 = (wsT[b][:, m, :]
                       .rearrange("p (t2 two) -> p t2 two", two=2)
                       [:, None, :, :].broadcast_to([128, scount, 32, 2]))
                oap = big[:, m, 0:64 * scount].rearrange(
                    "p (s t2 two) -> p s t2 two", s=scount, two=2)
                nc.vector.tensor_tensor(oap, in0, in1, op=mybir.AluOpType.add)
                # per-half tanh so PE can consume m=0 while m=1 still runs
                nc.scalar.activation(big[:, m, 0:64 * scount],
                                     big[:, m, 0:64 * scount],
                                     mybir.ActivationFunctionType.Tanh)
            # h-reduction: chunk k (8 s) -> psum rows 32*(k//2), col half k%2
            for k in range(nch):
                r, ch = 32 * (k // 2), 512 * (k % 2)
                for m in range(2):
                    nc.tensor.matmul(scq[r:r + 32, ch:ch + 512],
                                     vrep_sb[:, m, :],
                                     big[:, m, 512 * k:512 * (k + 1)],
                                     start=(m == 0), stop=(m == 1))

        def slab_evict(b, g, sbase, scount, scq):
            nch = scount // 8
            nrows = 32 * ((nch + 1) // 2)
            scr = smallp.tile([96, 1024], F32, tag="scr")
            use_if = g > 0 and not os.environ.get("KERNEL_NO_IF")
            if use_if:
                with tc.If(rv[b] > sbase) as cmp:
                    nc.vector.tensor_copy(scr[0:nrows, 0:512],
                                          scq[0:nrows, 0:512])
                    nc.scalar.activation(scr[0:nrows, 512:1024],
                                         scq[0:nrows, 512:1024],
                                         mybir.ActivationFunctionType.Copy)
                with cmp.Else():
                    # keep scr defined: relayout below copies it into masked
                    # (s >= src_len) rows
                    nc.vector.memset(scr[:], 0.0)
            else:
                nc.vector.tensor_copy(scr[0:nrows, 0:512], scq[0:nrows, 0:512])
                nc.scalar.activation(scr[0:nrows, 512:1024],
                                     scq[0:nrows, 512:1024],
                                     mybir.ActivationFunctionType.Copy)
            # relayout DMAs stay unconditional (skipped-branch DMAs would
            # leave their completion sems un-incremented -> device hang).
            for r in range((nch + 1) // 2):
                src = scr[32 * r:32 * r + 1, :].rearrange(
                    "p (h s t) -> p h s t", h=2, s=8)
                nc.sync.dma_start(
                    scoresT[b][sbase + 16 * r:sbase + 16 * r + 16, :], src)

        slabs = []
        sbase = 0
        for g, scount in enumerate(SGROUPS):
            for b in range(BPC):
                slabs.append((b, g, sbase, scount))
            sbase += scount

        pending = None
        for (b, g, sbase, scount) in slabs:
            scq = scq_ps.tile([128, 1024], F32, tag="scq",
                              name=f"scq{b}g{g}")
            if g == 0 or os.environ.get("KERNEL_NO_IF"):
                slab_compute(b, sbase, scount, scq)  # src_len>=1: always
            else:
                with tc.If(rv[b] > sbase) as cmp:
                    slab_compute(b, sbase, scount, scq)
            if pending is not None:
                slab_evict(*pending)
            pending = (b, g, sbase, scount, scq)
            # filler matmuls: keep PE HAM busy across the tanh-wait gap so
            # the clock stays at K=8/8 (idle >3.4us would re-throttle)
            if not os.environ.get("KERNEL_NO_WARM"):
                fill_ps = fc_ps.tile([32, 512], F32, tag="fc")
                for _ in range(3):
                    nc.tensor.matmul(
                        fill_ps, vrep_sb[:, 0, :],
                        w1_sb[:, 0:2, :].rearrange("p c m -> p (c m)"),
                        start=True, stop=True, skip_group_check=True)
        slab_evict(*pending)

        # ---- phase 3: masked exp + final matmul + normalize + store ----
        for b in range(BPC):
            e_sb = smallp.tile([S, T], F32, tag="e_sb")
            nc.scalar.activation(e_sb, scoresT[b],
                                 mybir.ActivationFunctionType.Exp,
                                 bias=maskb_sb[:, b:b + 1])
            # out[t,e] = sum_s e[s,t] outputs[s,e]; den via ones column
            ops = out_ps.tile([64, 520], F32, tag="ops")
            nc.tensor.matmul(ops[:, 512:513], e_sb, ones_sb,
                             start=True, stop=True)
            rden = smallp.tile([64, 1], F32, tag="rden")
            nc.vector.reciprocal(rden, ops[:, 512:513])
            nc.tensor.matmul(ops[:, 0:512], e_sb, outs_nat[b],
                             start=True, stop=True)
            res = work.tile([64, E], F32, tag="res")
            for h in range(2):
                cs = slice(h * 256, (h + 1) * 256)
                nc.vector.tensor_scalar_mul(res[:, cs], ops[:, cs], rden)
                nc.sync.dma_start(out_d[b][:, cs], res[:, cs])

    nc.finalize()
    return nc


def _get_program():
    if "nc" not in _prog_cache:
        _prog_cache["nc"] = build_program()
    return _prog_cache["nc"]


def _balanced_assignment(src_len):
    """Pair batches so per-core skipped work is balanced (min-max coverage).

    Returns perm: perm[c*BPC+j] = original batch index placed at core c slot j.
    """
    bounds = np.cumsum([0] + SGROUPS)  # 0, 48, 96, 128
    cov = [int(bounds[np.searchsorted(bounds, l)]) for l in src_len]
    order = sorted(range(B), key=lambda i: -cov[i])
    loads = [0] * NCORES
    slots = [[] for _ in range(NCORES)]
    for i in order:
        c = min((k for k in range(NCORES) if len(slots[k]) < BPC),
                key=lambda k: loads[k])
        slots[c].append(i)
        loads[c] += cov[i]
    return [i for sl in slots for i in sl]


def make_in_maps(outputs, src_len, ss, W1, b1, W2, b2, v_w, v_b):
    outputs = np.asarray(outputs, dtype=np.float32)
    ss = np.asarray(ss, dtype=np.float32)
    src_len = np.asarray(src_len).astype(np.int64)
    perm = _balanced_assignment(src_len)
    maskb = np.where(np.arange(S)[None, :] < src_len[:, None],
                     np.float32(0.0), np.float32(-1e30)).astype(np.float32)
    o_b = np.ascontiguousarray(outputs.transpose(1, 0, 2))  # (B, S, E)
    oT16 = np.ascontiguousarray(outputs.transpose(1, 2, 0).astype(np.float16))
    sT16 = np.ascontiguousarray(ss.transpose(1, 2, 0).astype(np.float16))
    w1_16 = np.asarray(W1, dtype=np.float16)
    w2_16 = np.asarray(W2, dtype=np.float16)
    vrep = np.repeat(np.asarray(v_w, dtype=np.float16).reshape(2, 128)
                     .transpose(1, 0)[:, :, None], 32, axis=2)  # [128, 2, 32]
    vrep = np.ascontiguousarray(vrep)
    b1_32 = np.asarray(b1, dtype=np.float32)
    b2_32 = np.asarray(b2, dtype=np.float32)
    in_maps = []
    for c in range(NCORES):
        idx = perm[c * BPC:(c + 1) * BPC]
        in_maps.append({
            "outs": np.ascontiguousarray(o_b[idx]),
            "outsT16": np.ascontiguousarray(oT16[idx]),
            "ssT16": np.ascontiguousarray(sT16[idx]),
            "w1": w1_16, "w2": w2_16, "b1": b1_32, "b2": b2_32,
            "vrep": vrep, "maskb": np.ascontiguousarray(maskb[idx]),
            "slen": np.ascontiguousarray(src_len[idx].astype(np.int32)),
        })
    return in_maps, perm


def run(in_maps, trace=False, **kw):
    nc = _get_program()
    return run_bass_kernel_spmd(nc, in_maps, list(range(NCORES)), trace=trace, **kw)


def kernel(outputs, src_len, ss, W1, b1, W2, b2, v_w, v_b):
    in_maps, perm = make_in_maps(outputs, src_len, ss, W1, b1, W2, b2, v_w, v_b)
    res = run(in_maps)
    shuffled = np.concatenate([np.asarray(r["out"]).reshape(BPC, T, E)
                               for r in res.results], axis=0)
    out = np.empty_like(shuffled)
    out[np.asarray(perm)] = shuffled
    return out.astype(np.float32)  # (B, T, 2H)

